# revision 8
# baseline (speedup 1.0000x reference)
"""Self-contained Trainium2 Bass kernel for a single attention head.

Problem: B=8, S=2048, E=1024, D=64 (fp32 in/out).
  q = query @ Wq.T + bq ; k, v likewise
  out = softmax(mask(q @ k.T / sqrt(D))) @ v
  mask = query_mask[:, :, None] * key_mask[:, None, :]; query_mask is all-ones
  per the problem spec (fill="ones").

Sharding: pure data-parallel, one batch element per NeuronCore (8 cores).

Design (v2):
  - fp16 compute with fp32 PSUM accumulation for proj/AV; scores go to
    fp16 PSUM tiles (1 bank each) read directly by the ACT exp.
  - Host compacts away masked key columns; S_k shrinks 2048 -> ~1100,
    padded to a multiple of 128; pad columns get exp bias -30000 -> 0.
  - q/k projections use a column-duplicated stationary [W|W] (M=128,
    same cycle cost as M=64) so qT/kT land in BOTH partition halves.
    That enables row-tiled CONCURRENT score matmul pairs: chunk c0 runs
    in PE rows 0-63 (kT lo) while chunk c1 runs in rows 64-127 (kT hi),
    tile_position auto-derived from base partitions -> ~2x score rate.
  - (q2,q3) projection is column-PAIRED: two concurrent M=64 matmuls in
    PE col groups (out partitions 0-63 / 64-127) per e-pass -> half-1 q
    projected in half the passes; h1 chunk c0 reads the lo rows, c1 hi.
  - ~12 warm-up matmuls on junk data during the DMA ramp keep the PE
    HAM clock at 2.4 GHz so the first real projections run warm.
  - DMA order matches the exp chain's consumption order: weights, q0,
    k0, q1, k1, k2, q2q3, v0, v1, v2 as fat whole-piece transfers.
  - PSUM: 2-slot fp16 ssT ring (2 banks) + 2-slot proj/transpose pool
    (2) + num0 h0 accumulator (2) + num1 h1 chunk accumulators (2) = 8.
    Separate h1 accumulators let AV h1 chase each exp immediately
    instead of waiting for the h0 AV chain to drain.
  - Emission order keeps h1 scores+exps ahead of all AV/v work so the
    ACT exp chain never starves; the AV tail after the last exp is just
    2 matmuls + chunked copies (split ACT/DVE) + 2 chunked stores.
  - No on-chip normalize/transpose finale: raw [65, S] numerator rows
    go PSUM -> SBUF fp16 -> DRAM; the host does (num[:64]/num[64]).T.
"""

from contextlib import ExitStack

import numpy as np

import concourse.bass as bass
import concourse.mybir as mybir
import concourse.tile as tile
from concourse import bacc
from concourse.bass_utils import run_bass_kernel_spmd
from concourse.masks import make_identity

FP16 = mybir.dt.float16
F32 = mybir.dt.float32

N_CORES = 8
B, S, E, D = 8, 2048, 1024, 64
P = 128
NE = E // P            # 8 contraction tiles
NH = 2                 # query halves
HI = S // NH           # 1024 query positions per half
NC = 512               # matmul free-dim chunk (one PSUM bank of f32)
SCALE = 1.0 / np.sqrt(np.float32(D))
MASK_NEG = -30000.0
N_WARM = 12


def _chunks(total, step, base=0):
    out = []
    o = 0
    while o < total:
        out.append((base + o, min(step, total - o)))
        o += step
    return out


def _build(tc: tile.TileContext, ins: dict, out_d: bass.AP, ctx, sk2: int,
           nkr: int):
    nc = tc.nc
    nj = sk2 // P
    kp = _chunks(nkr, NC)
    vp = _chunks(nkr, NC)

    consts = ctx.enter_context(tc.tile_pool(name="consts", bufs=1))
    stage = ctx.enter_context(tc.tile_pool(name="stage", bufs=1))
    proj = ctx.enter_context(tc.tile_pool(name="proj", bufs=1))
    xpool = ctx.enter_context(tc.tile_pool(name="xpool", bufs=max(nj, 2)))
    ppool = ctx.enter_context(tc.tile_pool(name="ppool", bufs=max(2 * nj, 2)))
    fin = ctx.enter_context(tc.tile_pool(name="fin", bufs=1))
    ps_mm = ctx.enter_context(tc.tile_pool(name="ps_mm", bufs=2, space="PSUM"))
    ps_sm = ctx.enter_context(tc.tile_pool(name="ps_sm", bufs=2, space="PSUM"))
    ps_acc = ctx.enter_context(tc.tile_pool(name="ps_acc", bufs=1,
                                            space="PSUM"))

    # --- staged inputs, HWDGE SP ring, consumption-deadline order --------
    wqd = consts.tile([P, NE * P], FP16, tag="wqd")
    wkv = consts.tile([P, NE * P + NE * D], FP16, tag="wkv")
    c32 = consts.tile([P, nj + 3], F32, tag="c32")
    q0s = stage.tile([P, NE * NC], FP16, tag="q0s")
    q1s = stage.tile([P, NE * NC], FP16, tag="q1s")
    q23s = stage.tile([P, NE * 2 * NC], FP16, tag="q23s")
    ksh = {i: stage.tile([P, NE * kp[i][1]], FP16, tag=f"k{i}",
                         name=f"ks{i}") for i in range(len(kp))}
    vsh = {i: stage.tile([P, NE * vp[i][1]], FP16, tag=f"v{i}",
                         name=f"vs{i}") for i in range(len(vp))}

    nc.sync.dma_start(out=wqd[:], in_=ins["wqd"][:])
    nc.sync.dma_start(out=wkv[:], in_=ins["wkv"][:])
    nc.sync.dma_start(out=c32[:], in_=ins["c32"][:])
    nc.sync.dma_start(out=q0s[:], in_=ins["q0"][:])
    nc.sync.dma_start(out=ksh[0][:], in_=ins["k0"][:])
    nc.sync.dma_start(out=q1s[:], in_=ins["q1"][:])
    for i in range(1, len(kp)):
        nc.sync.dma_start(out=ksh[i][:], in_=ins[f"k{i}"][:])
    nc.sync.dma_start(out=q23s[:], in_=ins["q23"][:])
    for i in range(len(vp)):
        nc.sync.dma_start(out=vsh[i][:], in_=ins[f"v{i}"][:])

    wkd = wkv[:, 0:NE * P]
    wv = wkv[:, NE * P:NE * P + NE * D]
    mb = c32[:, 0:nj]
    bq = c32[:, nj:nj + 1]          # duplicated rows 0-63 / 64-127
    bk = c32[:, nj + 1:nj + 2]
    bv = c32[0:D, nj + 2:nj + 3]

    # --- engine warm-up / constants --------------------------------------
    ident = consts.tile([P, P], FP16, tag="ident")
    junk = consts.tile([P, NC], FP16, tag="junk")
    warm = consts.tile([P, 16], F32, tag="warm")
    make_identity(nc, ident[:])
    nc.vector.memset(junk[:], 0.0)
    nc.vector.memset(warm[:], 0.0)
    nc.scalar.activation(warm[:], warm[:], mybir.ActivationFunctionType.Exp)

    # persistent projected tensors
    # qT128: cols 0:HI = half0 duplicated in both partition halves;
    #        cols HI:HI+NC = half1 (lo rows = q cols 1024-1536, hi rows =
    #        q cols 1536-2048) from the column-paired (q2,q3) projection.
    qT128 = proj.tile([P, HI + NC], FP16, tag="qT128")
    kT128 = proj.tile([P, sk2], FP16, tag="kT128")
    vT65 = proj.tile([D + 1, sk2], FP16, tag="vT65")
    nc.vector.memset(vT65[D:D + 1, :], 1.0)   # ones row -> softmax denom
    if nkr < sk2:
        nc.vector.memset(kT128[:, nkr:sk2], 0.0)
        nc.vector.memset(vT65[0:D, nkr:sk2], 0.0)

    # num0 allocated first so warm-up matmuls can target its PSUM.
    num0 = ps_acc.tile([D + 1, HI], F32, tag="num", name="num0")
    for w in range(N_WARM):
        nc.tensor.matmul(num0[0:D + 1, 0:NC], ident[:, 0:D + 1], junk[:],
                         start=True, stop=True, skip_group_check=True)

    # ---- projection helpers ---------------------------------------------
    def proj_dup(dst, dstcol, wd, bias_ap, src, n):
        """Duplicated-stationary projection: out [128, n], rows 64-127 a
        copy of rows 0-63."""
        ps = ps_sm.tile([P, NC], F32, tag="ps_sm", name=f"psd_{dstcol}")
        for e in range(NE):
            nc.tensor.matmul(
                ps[:, 0:n],
                wd[:, e * P:(e + 1) * P],
                src[:, e * n:(e + 1) * n],
                start=(e == 0), stop=(e == NE - 1),
            )
        nc.vector.tensor_scalar_add(dst[:, dstcol:dstcol + n], ps[:, 0:n],
                                    bias_ap)

    def proj_pair(dst, dstcol, wd, bias_ap, src, n):
        """Column-paired projection: two concurrent M=64 matmuls per
        e-pass; lo rows of dst get piece a (src cols 0:n per e), hi rows
        piece b."""
        ps = ps_sm.tile([P, NC], F32, tag="ps_sm", name=f"psp_{dstcol}")
        for e in range(NE):
            nc.tensor.matmul(
                ps[0:D, 0:n],
                wd[:, e * P:e * P + D],
                src[:, e * 2 * n:e * 2 * n + n],
                start=(e == 0), stop=(e == NE - 1),
            )
            nc.tensor.matmul(
                ps[D:P, 0:n],
                wd[:, e * P:e * P + D],
                src[:, e * 2 * n + n:(e + 1) * 2 * n],
                start=(e == 0), stop=(e == NE - 1),
            )
        nc.vector.tensor_scalar_add(dst[0:D, dstcol:dstcol + n],
                                    ps[0:D, 0:n], bias_ap[0:D])
        nc.vector.tensor_scalar_add(dst[D:P, dstcol:dstcol + n],
                                    ps[D:P, 0:n], bias_ap[D:P])

    def proj_v(dstcol, src, n):
        ps = ps_sm.tile([P, NC], F32, tag="ps_sm", name=f"psv_{dstcol}")
        for e in range(NE):
            nc.tensor.matmul(
                ps[0:D, 0:n],
                wv[:, e * D:(e + 1) * D],
                src[:, e * n:(e + 1) * n],
                start=(e == 0), stop=(e == NE - 1),
            )
        nc.vector.tensor_scalar_add(vT65[0:D, dstcol:dstcol + n],
                                    ps[0:D, 0:n], bv)

    # ---- attention helpers ----------------------------------------------
    pms = {}

    def sc(h, j):
        """Scores for (h, j): two row-tiled concurrent N=512 matmuls into
        one fp16 PSUM bank, then the exp into SBUF."""
        sst = ps_mm.tile([P, HI], F32, tag="ps_mm", name=f"ssT_{h}_{j}")
        if h == 0:
            qlo = qT128[0:D, 0:NC]
            qhi = qT128[D:P, NC:HI]
        else:
            qlo = qT128[0:D, HI:HI + NC]
            qhi = qT128[D:P, HI:HI + NC]
        nc.tensor.matmul(sst[:, 0:NC], kT128[0:D, j * P:(j + 1) * P], qlo,
                         start=True, stop=True)
        nc.tensor.matmul(sst[:, NC:HI], kT128[D:P, j * P:(j + 1) * P], qhi,
                         start=True, stop=True)
        p = ppool.tile([P, HI], FP16, tag="pm", name=f"pm_{h}_{j}")
        nc.scalar.activation(p[:], sst[:], mybir.ActivationFunctionType.Exp,
                             bias=mb[:, j:j + 1], scale=float(SCALE))
        pms[(h, j)] = p

    xt = [None] * nj

    def x_one(j):
        pst = ps_sm.tile([P, D + 1], FP16, tag="ps_sm", name=f"psx{j}")
        nc.tensor.transpose(pst[:], vT65[:, j * P:(j + 1) * P],
                            ident[0:D + 1, 0:D + 1])
        x = xpool.tile([P, D + 1], FP16, tag="x", name=f"x{j}")
        nc.vector.tensor_copy(x[:], pst[:])
        xt[j] = x

    def av0(j):
        for c in range(HI // NC):
            nc.tensor.matmul(
                num0[:, c * NC:(c + 1) * NC],
                xt[j][:],
                pms[(0, j)][:, c * NC:(c + 1) * NC],
                start=(j == 0), stop=(j == nj - 1),
            )

    # h1 chunk accumulators rotate through the ps_sm pool, which is idle
    # once the last transpose drains. They MUST be the pool's final
    # allocations (rotation order == usage order), so allocate lazily.
    numc = []

    def av1(j):
        if not numc:
            numc.extend(
                ps_sm.tile([D + 1, NC], F32, tag="ps_sm", name=f"numc{c}")
                for c in range(HI // NC))
        for c in range(HI // NC):
            nc.tensor.matmul(
                numc[c][:],
                xt[j][:],
                pms[(1, j)][:, c * NC:(c + 1) * NC],
                start=(j == 0), stop=(j == nj - 1),
            )

    def pv(i):
        proj_v(vp[i][0], vsh[i][:], vp[i][1])

    # ---- emission --------------------------------------------------------
    proj_dup(qT128, 0, wqd, bq, q0s[:], NC)
    proj_dup(kT128, 0, wkd, bk, ksh[0][:], kp[0][1])
    proj_dup(qT128, NC, wqd, bq, q1s[:], NC)
    sc(0, 0)
    sc(0, 1)
    if len(kp) > 1:
        proj_dup(kT128, kp[1][0], wkd, bk, ksh[1][:], kp[1][1])
    for j in range(2, min(4, nj)):
        sc(0, j)
    for i in range(2, len(kp)):
        proj_dup(kT128, kp[i][0], wkd, bk, ksh[i][:], kp[i][1])
    for j in range(4, nj):
        sc(0, j)
    proj_pair(qT128, HI, wqd, bq, q23s[:], NC)

    # h1 scores interleaved with v/AV work so the PE has useful work
    # while each h1 score waits for its ssT ring slot (exp-paced).
    if nj == 9 and len(vp) == 3:
        sc(1, 0)
        sc(1, 1)
        pv(0)
        sc(1, 2)
        x_one(0)
        x_one(1)
        sc(1, 3)
        x_one(2)
        x_one(3)
        av0(0)
        sc(1, 4)
        av0(1)
        sc(1, 5)
        av0(2)
        sc(1, 6)
        pv(1)
        av0(3)
        sc(1, 7)
        x_one(4)
        x_one(5)
        av0(4)
        sc(1, 8)
        x_one(6)
        x_one(7)
        av0(5)
        av0(6)
        pv(2)
        x_one(8)
        av0(7)
        av0(8)
    else:
        for j in range(nj):
            sc(1, j)
        done_x = 0
        for i, (o, n) in enumerate(vp):
            pv(i)
            hi_j = nj if i == len(vp) - 1 else (o + n) // P
            for j in range(done_x, hi_j):
                x_one(j)
                av0(j)
            done_x = hi_j

    nsb0 = fin.tile([D + 1, HI], FP16, tag="nsb0")
    nc.vector.tensor_copy(nsb0[:], num0[:])
    nc.sync.dma_start(out=out_d[0:D + 1, :], in_=nsb0[:])

    nsb1 = fin.tile([D + 1, HI], FP16, tag="nsb1")
    for j in range(nj):
        av1(j)
    # tail: chunk c0 copy on ACT (idle after the last exp), c1 on DVE,
    # stores chunked so the first can fly while the second copies.
    nc.scalar.copy(nsb1[:, 0:NC], numc[0][:])
    nc.sync.dma_start(out=out_d[D + 1:2 * (D + 1), 0:NC],
                      in_=nsb1[:, 0:NC])
    nc.vector.tensor_copy(nsb1[:, NC:HI], numc[1][:])
    nc.sync.dma_start(out=out_d[D + 1:2 * (D + 1), NC:HI],
                      in_=nsb1[:, NC:HI])


_COMPILED = {}


def _get_compiled(sk2: int, nkr: int):
    key = (sk2, nkr)
    if key not in _COMPILED:
        nj = sk2 // P
        kp = _chunks(nkr, NC)
        vp = _chunks(nkr, NC)
        nc = bacc.Bacc("TRN2", target_bir_lowering=False, debug=False,
                       num_devices=N_CORES)

        def din(name, shape, dt=FP16):
            return nc.dram_tensor(name, shape, dt, kind="ExternalInput").ap()

        ins = {"wqd": din("wqd", [P, NE * P]),
               "wkv": din("wkv", [P, NE * P + NE * D]),
               "c32": din("c32", [P, nj + 3], F32),
               "q0": din("q0", [P, NE * NC]),
               "q1": din("q1", [P, NE * NC]),
               "q23": din("q23", [P, NE * 2 * NC])}
        for pref, pieces in (("k", kp), ("v", vp)):
            for i, (o, n) in enumerate(pieces):
                ins[f"{pref}{i}"] = din(f"{pref}{i}", [P, NE * n])
        out_d = nc.dram_tensor("out", [NH * (D + 1), HI], FP16,
                               kind="ExternalOutput").ap()
        with tile.TileContext(nc) as tc:
            with ExitStack() as ctx:
                _build(tc, ins, out_d, ctx, sk2, nkr)
        nc.compile()
        _COMPILED[key] = nc
    return _COMPILED[key]


def _blob(x16, lo, hi):
    """[S', E] fp16 row-slice -> staging blob [P, NE*(hi-lo)] laid out as
    [partition, e-block, col]."""
    return np.ascontiguousarray(
        x16[lo:hi].reshape(hi - lo, NE, P).transpose(2, 1, 0)
    ).reshape(P, -1)


LAST_RESULTS = None


def kernel(query, key, value, query_mask, key_mask, Wq, bq, Wk, bk, Wv, bv):
    global LAST_RESULTS
    query = np.asarray(query, dtype=np.float32)
    key = np.asarray(key, dtype=np.float32)
    value = np.asarray(value, dtype=np.float32)
    key_mask = np.asarray(key_mask)

    # compact masked keys away (they contribute exactly zero)
    keeps = [np.nonzero(key_mask[c] != 0)[0] for c in range(N_CORES)]
    nk_max = max(len(kps) for kps in keeps)
    sk2 = max(P, int(np.ceil(nk_max / P)) * P)
    sk2 = min(sk2, S)
    nkr = min(sk2, max(P, int(np.ceil(nk_max / 64)) * 64))
    nj = sk2 // P
    kp = _chunks(nkr, NC)
    vp = _chunks(nkr, NC)

    def wblob(w):
        return (np.asarray(w, np.float32).astype(np.float16)
                .reshape(D, NE, P).transpose(2, 1, 0).reshape(P, NE, D))

    wq3 = wblob(Wq)
    wqd = np.concatenate([wq3, wq3], axis=2).reshape(P, NE * P)
    wk3 = wblob(Wk)
    wkd = np.concatenate([wk3, wk3], axis=2).reshape(P, NE * P)
    wv2 = wblob(Wv).reshape(P, NE * D)
    wkv = np.ascontiguousarray(np.concatenate([wkd, wv2], axis=1))

    c32 = np.zeros((P, nj + 3), np.float32)
    for i, b in enumerate((bq, bk)):
        bb = np.asarray(b, np.float32).reshape(D)
        c32[0:D, nj + i] = bb
        c32[D:P, nj + i] = bb
    c32[0:D, nj + 2] = np.asarray(bv, np.float32).reshape(D)

    in_maps = []
    for c in range(N_CORES):
        kps = keeps[c]
        nk = len(kps)
        q16 = query[c].astype(np.float16)
        kc = np.zeros((nkr, E), np.float16)
        vc = np.zeros((nkr, E), np.float16)
        kc[0:nk] = key[c][kps].astype(np.float16)
        vc[0:nk] = value[c][kps].astype(np.float16)
        c32c = c32.copy()
        mbias = np.full(sk2, np.float32(MASK_NEG))
        mbias[0:nk] = 0.0
        c32c[:, 0:nj] = mbias.reshape(nj, P).T
        # q23 blob: [P, NE, 2*NC] with per-e layout [piece2 | piece3]
        b2 = _blob(q16, HI, HI + NC).reshape(P, NE, NC)
        b3 = _blob(q16, HI + NC, S).reshape(P, NE, NC)
        q23 = np.ascontiguousarray(
            np.concatenate([b2, b3], axis=2)).reshape(P, -1)
        im = {"wqd": wqd, "wkv": wkv, "c32": np.ascontiguousarray(c32c),
              "q0": _blob(q16, 0, NC), "q1": _blob(q16, NC, HI),
              "q23": q23}
        for pref, pieces, arr in (("k", kp, kc), ("v", vp, vc)):
            for i, (o, n) in enumerate(pieces):
                im[f"{pref}{i}"] = _blob(arr, o, o + n)
        in_maps.append(im)

    nc = _get_compiled(sk2, nkr)
    res = run_bass_kernel_spmd(nc, in_maps, core_ids=list(range(N_CORES)))
    LAST_RESULTS = res

    out = np.empty((N_CORES, S, D), np.float32)
    for c in range(N_CORES):
        o = np.asarray(res.results[c]["out"]).astype(np.float32)
        for h in range(NH):
            nh = o[h * (D + 1):(h + 1) * (D + 1)]
            out[c, h * HI:(h + 1) * HI] = (nh[0:D] / nh[D:D + 1]).T
    return out


# revision 10
# speedup vs baseline: 1.0365x; 1.0365x over previous
"""Self-contained Trainium2 Bass kernel for a single attention head.

Problem: B=8, S=2048, E=1024, D=64 (fp32 in/out).
  q = query @ Wq.T + bq ; k, v likewise
  out = softmax(mask(q @ k.T / sqrt(D))) @ v
  mask = query_mask[:, :, None] * key_mask[:, None, :]; query_mask is all-ones
  per the problem spec (fill="ones").

Sharding: pure data-parallel, one batch element per NeuronCore (8 cores).

Design (v2):
  - fp16 compute with fp32 PSUM accumulation for proj/AV; scores go to
    fp16 PSUM tiles (1 bank each) read directly by the ACT exp.
  - Host compacts away masked key columns; S_k shrinks 2048 -> ~1100,
    padded to a multiple of 128; pad columns get exp bias -30000 -> 0.
  - q/k projections use a column-duplicated stationary [W|W] (M=128,
    same cycle cost as M=64) so qT/kT land in BOTH partition halves.
    That enables row-tiled CONCURRENT score matmul pairs: chunk c0 runs
    in PE rows 0-63 (kT lo) while chunk c1 runs in rows 64-127 (kT hi),
    tile_position auto-derived from base partitions -> ~2x score rate.
  - (q2,q3) projection is column-PAIRED: two concurrent M=64 matmuls in
    PE col groups (out partitions 0-63 / 64-127) per e-pass -> half-1 q
    projected in half the passes; h1 chunk c0 reads the lo rows, c1 hi.
  - ~12 warm-up matmuls on junk data during the DMA ramp keep the PE
    HAM clock at 2.4 GHz so the first real projections run warm.
  - DMA order matches the exp chain's consumption order: weights, q0,
    k0, q1, k1, k2, q2q3, v0, v1, v2 as fat whole-piece transfers.
  - PSUM: 2-slot fp16 ssT ring (2 banks) + 2-slot proj/transpose pool
    (2) + num0 h0 accumulator (2) + num1 h1 chunk accumulators (2) = 8.
    Separate h1 accumulators let AV h1 chase each exp immediately
    instead of waiting for the h0 AV chain to drain.
  - Emission order keeps h1 scores+exps ahead of all AV/v work so the
    ACT exp chain never starves; the AV tail after the last exp is just
    2 matmuls + chunked copies (split ACT/DVE) + 2 chunked stores.
  - No on-chip normalize/transpose finale: raw [65, S] numerator rows
    go PSUM -> SBUF fp16 -> DRAM; the host does (num[:64]/num[64]).T.
"""

from contextlib import ExitStack

import numpy as np

import concourse.bass as bass
import concourse.mybir as mybir
import concourse.tile as tile
from concourse import bacc
from concourse.bass_utils import run_bass_kernel_spmd
from concourse.masks import make_identity

FP16 = mybir.dt.float16
F32 = mybir.dt.float32

N_CORES = 8
B, S, E, D = 8, 2048, 1024, 64
P = 128
NE = E // P            # 8 contraction tiles
NH = 2                 # query halves
HI = S // NH           # 1024 query positions per half
NC = 512               # matmul free-dim chunk (one PSUM bank of f32)
SCALE = 1.0 / np.sqrt(np.float32(D))
MASK_NEG = -30000.0
N_WARM = 22


def _chunks(total, step, base=0):
    out = []
    o = 0
    while o < total:
        out.append((base + o, min(step, total - o)))
        o += step
    return out


def _build(tc: tile.TileContext, ins: dict, out_d: bass.AP, ctx, sk2: int,
           nkr: int):
    nc = tc.nc
    nj = sk2 // P
    kp = _chunks(nkr, NC)
    vp = _chunks(nkr, NC)

    consts = ctx.enter_context(tc.tile_pool(name="consts", bufs=1))
    stage = ctx.enter_context(tc.tile_pool(name="stage", bufs=1))
    proj = ctx.enter_context(tc.tile_pool(name="proj", bufs=1))
    xpool = ctx.enter_context(tc.tile_pool(name="xpool", bufs=max(nj, 2)))
    ppool = ctx.enter_context(tc.tile_pool(name="ppool", bufs=max(2 * nj, 2)))
    fin = ctx.enter_context(tc.tile_pool(name="fin", bufs=1))
    ps_mm = ctx.enter_context(tc.tile_pool(name="ps_mm", bufs=2, space="PSUM"))
    ps_sm = ctx.enter_context(tc.tile_pool(name="ps_sm", bufs=2, space="PSUM"))
    ps_acc = ctx.enter_context(tc.tile_pool(name="ps_acc", bufs=1,
                                            space="PSUM"))

    # --- staged inputs, HWDGE SP ring, consumption-deadline order --------
    wqd = consts.tile([P, NE * P], FP16, tag="wqd")
    wkv = consts.tile([P, NE * P + NE * D], FP16, tag="wkv")
    c32 = consts.tile([P, nj + 3], F32, tag="c32")
    q0s = stage.tile([P, NE * NC], FP16, tag="q0s")
    q1s = stage.tile([P, NE * NC], FP16, tag="q1s")
    q23s = stage.tile([P, NE * 2 * NC], FP16, tag="q23s")
    ksh = {i: stage.tile([P, NE * kp[i][1]], FP16, tag=f"k{i}",
                         name=f"ks{i}") for i in range(len(kp))}
    vsh = {i: stage.tile([P, NE * vp[i][1]], FP16, tag=f"v{i}",
                         name=f"vs{i}") for i in range(len(vp))}

    nc.sync.dma_start(out=wqd[:], in_=ins["wqd"][:])
    nc.sync.dma_start(out=wkv[:], in_=ins["wkv"][:])
    nc.sync.dma_start(out=c32[:], in_=ins["c32"][:])
    nc.sync.dma_start(out=q0s[:], in_=ins["q0"][:])
    nc.sync.dma_start(out=ksh[0][:], in_=ins["k0"][:])
    nc.sync.dma_start(out=q1s[:], in_=ins["q1"][:])
    for i in range(1, len(kp)):
        nc.sync.dma_start(out=ksh[i][:], in_=ins[f"k{i}"][:])
    nc.sync.dma_start(out=q23s[:], in_=ins["q23"][:])
    for i in range(len(vp)):
        nc.sync.dma_start(out=vsh[i][:], in_=ins[f"v{i}"][:])

    wkd = wkv[:, 0:NE * P]
    wv = wkv[:, NE * P:NE * P + NE * D]
    mb = c32[:, 0:nj]
    bq = c32[:, nj:nj + 1]          # duplicated rows 0-63 / 64-127
    bk = c32[:, nj + 1:nj + 2]
    bv = c32[0:D, nj + 2:nj + 3]

    # --- engine warm-up / constants --------------------------------------
    ident = consts.tile([P, P], FP16, tag="ident")
    junk = consts.tile([P, NC], FP16, tag="junk")
    warm = consts.tile([P, 16], F32, tag="warm")
    make_identity(nc, ident[:])
    nc.vector.memset(junk[:], 0.0)
    nc.vector.memset(warm[:], 0.0)
    nc.scalar.activation(warm[:], warm[:], mybir.ActivationFunctionType.Exp)

    # persistent projected tensors
    # qT128: cols 0:HI = half0 duplicated in both partition halves;
    #        cols HI:HI+NC = half1 (lo rows = q cols 1024-1536, hi rows =
    #        q cols 1536-2048) from the column-paired (q2,q3) projection.
    qT128 = proj.tile([P, HI + NC], FP16, tag="qT128")
    kT128 = proj.tile([P, sk2], FP16, tag="kT128")
    vT65 = proj.tile([D + 1, sk2], FP16, tag="vT65")
    nc.vector.memset(vT65[D:D + 1, :], 1.0)   # ones row -> softmax denom
    if nkr < sk2:
        nc.vector.memset(kT128[:, nkr:sk2], 0.0)
        nc.vector.memset(vT65[0:D, nkr:sk2], 0.0)

    # num0 allocated first so warm-up matmuls can target its PSUM.
    num0 = ps_acc.tile([D + 1, HI], F32, tag="num", name="num0")
    for w in range(N_WARM):
        nc.tensor.matmul(num0[0:D + 1, 0:NC], ident[:, 0:D + 1], junk[:],
                         start=True, stop=True, skip_group_check=True)

    # ---- projection helpers ---------------------------------------------
    def add_split(dst, ps_ap, bias_ap, n):
        """Bias add split across DVE + ACT halves (halves the latency on
        the exp-chain critical path; ACT is idle before the first exp)."""
        h = n // 2
        nc.vector.tensor_scalar_add(dst[:, 0:h], ps_ap[:, 0:h], bias_ap)
        nc.scalar.add(dst[:, h:n], ps_ap[:, h:n], bias_ap)

    def proj_dup(dst, dstcol, wd, bias_ap, src, n, fast_add=False):
        """Duplicated-stationary projection: out [128, n], rows 64-127 a
        copy of rows 0-63."""
        ps = ps_sm.tile([P, NC], F32, tag="ps_sm", name=f"psd_{dstcol}")
        for e in range(NE):
            nc.tensor.matmul(
                ps[:, 0:n],
                wd[:, e * P:(e + 1) * P],
                src[:, e * n:(e + 1) * n],
                start=(e == 0), stop=(e == NE - 1),
            )
        if fast_add:
            add_split(dst[:, dstcol:dstcol + n], ps[:, 0:n], bias_ap, n)
        else:
            nc.vector.tensor_scalar_add(dst[:, dstcol:dstcol + n],
                                        ps[:, 0:n], bias_ap)

    class ProjPair:
        """Column-paired projection, emitted in e-pass chunks so passes
        interleave into exp-paced PE stall gaps."""

        def __init__(self, dst, dstcol, wd, bias_ap, src, n):
            self.__dict__.update(dst=dst, dstcol=dstcol, wd=wd,
                                 bias_ap=bias_ap, src=src, n=n)
            self.ps = ps_sm.tile([P, NC], F32, tag="ps_sm",
                                 name=f"psp_{dstcol}")

        def passes(self, e0, e1):
            n = self.n
            for e in range(e0, e1):
                nc.tensor.matmul(
                    self.ps[0:D, 0:n],
                    self.wd[:, e * P:e * P + D],
                    self.src[:, e * 2 * n:e * 2 * n + n],
                    start=(e == 0), stop=(e == NE - 1),
                )
                nc.tensor.matmul(
                    self.ps[D:P, 0:n],
                    self.wd[:, e * P:e * P + D],
                    self.src[:, e * 2 * n + n:(e + 1) * 2 * n],
                    start=(e == 0), stop=(e == NE - 1),
                )

        def adds(self):
            n, dstcol = self.n, self.dstcol
            nc.vector.tensor_scalar_add(
                self.dst[0:D, dstcol:dstcol + n], self.ps[0:D, 0:n],
                self.bias_ap[0:D])
            nc.vector.tensor_scalar_add(
                self.dst[D:P, dstcol:dstcol + n], self.ps[D:P, 0:n],
                self.bias_ap[D:P])

    class ProjV:
        def __init__(self, dstcol, src, n):
            self.__dict__.update(dstcol=dstcol, src=src, n=n)
            self.ps = ps_sm.tile([P, NC], F32, tag="ps_sm",
                                 name=f"psv_{dstcol}")

        def passes(self, e0, e1):
            n = self.n
            for e in range(e0, e1):
                nc.tensor.matmul(
                    self.ps[0:D, 0:n],
                    wv[:, e * D:(e + 1) * D],
                    self.src[:, e * n:(e + 1) * n],
                    start=(e == 0), stop=(e == NE - 1),
                )

        def adds(self):
            nc.vector.tensor_scalar_add(
                vT65[0:D, self.dstcol:self.dstcol + self.n],
                self.ps[0:D, 0:self.n], bv)

    # ---- attention helpers ----------------------------------------------
    pms = {}

    def sc(h, j):
        """Scores for (h, j): two row-tiled concurrent N=512 matmuls into
        one fp16 PSUM bank, then the exp into SBUF."""
        sst = ps_mm.tile([P, HI], F32, tag="ps_mm", name=f"ssT_{h}_{j}")
        if h == 0:
            qlo = qT128[0:D, 0:NC]
            qhi = qT128[D:P, NC:HI]
        else:
            qlo = qT128[0:D, HI:HI + NC]
            qhi = qT128[D:P, HI:HI + NC]
        nc.tensor.matmul(sst[:, 0:NC], kT128[0:D, j * P:(j + 1) * P], qlo,
                         start=True, stop=True)
        nc.tensor.matmul(sst[:, NC:HI], kT128[D:P, j * P:(j + 1) * P], qhi,
                         start=True, stop=True)
        p = ppool.tile([P, HI], FP16, tag="pm", name=f"pm_{h}_{j}")
        nc.scalar.activation(p[:], sst[:], mybir.ActivationFunctionType.Exp,
                             bias=mb[:, j:j + 1], scale=float(SCALE))
        pms[(h, j)] = p

    xt = [None] * nj

    def x_one(j):
        pst = ps_sm.tile([P, D + 1], FP16, tag="ps_sm", name=f"psx{j}")
        nc.tensor.transpose(pst[:], vT65[:, j * P:(j + 1) * P],
                            ident[0:D + 1, 0:D + 1])
        x = xpool.tile([P, D + 1], FP16, tag="x", name=f"x{j}")
        nc.vector.tensor_copy(x[:], pst[:])
        xt[j] = x

    def av0(j):
        for c in range(HI // NC):
            nc.tensor.matmul(
                num0[:, c * NC:(c + 1) * NC],
                xt[j][:],
                pms[(0, j)][:, c * NC:(c + 1) * NC],
                start=(j == 0), stop=(j == nj - 1),
            )

    # h1 chunk accumulators rotate through the ps_sm pool, which is idle
    # once the last transpose drains. They MUST be the pool's final
    # allocations (rotation order == usage order), so allocate lazily.
    numc = []

    def av1(j):
        if not numc:
            numc.extend(
                ps_sm.tile([D + 1, NC], F32, tag="ps_sm", name=f"numc{c}")
                for c in range(HI // NC))
        for c in range(HI // NC):
            nc.tensor.matmul(
                numc[c][:],
                xt[j][:],
                pms[(1, j)][:, c * NC:(c + 1) * NC],
                start=(j == 0), stop=(j == nj - 1),
            )

    def pv(i):
        p = ProjV(vp[i][0], vsh[i][:], vp[i][1])
        p.passes(0, NE)
        p.adds()

    # ---- emission --------------------------------------------------------
    # Static order matters: every engine queue is FIFO, so a blocked
    # instruction stalls everything behind it on that engine. Scores are
    # exp-paced (~1.06us per ssT ring slot); filler work (later
    # projections, transposes, AV accumulation) is placed BEFORE each
    # blocking score so the PE spends the wait productively.
    proj_dup(qT128, 0, wqd, bq, q0s[:], NC, fast_add=True)
    proj_dup(kT128, 0, wkd, bk, ksh[0][:], kp[0][1], fast_add=True)
    proj_dup(qT128, NC, wqd, bq, q1s[:], NC, fast_add=True)
    sc(0, 0)
    sc(0, 1)
    if len(kp) > 1:
        proj_dup(kT128, kp[1][0], wkd, bk, ksh[1][:], kp[1][1])
    for j in range(2, min(4, nj)):
        sc(0, j)
    for i in range(2, len(kp)):
        proj_dup(kT128, kp[i][0], wkd, bk, ksh[i][:], kp[i][1])

    nsb0 = fin.tile([D + 1, HI], FP16, tag="nsb0")
    nsb1 = fin.tile([D + 1, HI], FP16, tag="nsb1")

    if nj == 9 and len(vp) == 3:
        q23 = ProjPair(qT128, HI, wqd, bq, q23s[:], NC)
        sc(0, 4)
        q23.passes(0, 3)
        sc(0, 5)
        q23.passes(3, 6)
        sc(0, 6)
        q23.passes(6, 8)
        q23.adds()
        sc(0, 7)
        v0 = ProjV(vp[0][0], vsh[0][:], vp[0][1])
        v0.passes(0, 4)
        sc(0, 8)
        v0.passes(4, 8)
        v0.adds()
        sc(1, 0)
        x_one(0)
        x_one(1)
        x_one(2)
        x_one(3)
        sc(1, 1)
        av0(0)
        av0(1)
        sc(1, 2)
        av0(2)
        av0(3)
        sc(1, 3)
        v1 = ProjV(vp[1][0], vsh[1][:], vp[1][1])
        v1.passes(0, 4)
        sc(1, 4)
        v1.passes(4, 8)
        v1.adds()
        sc(1, 5)
        x_one(4)
        x_one(5)
        x_one(6)
        x_one(7)
        av0(4)
        sc(1, 6)
        av0(5)
        v2 = ProjV(vp[2][0], vsh[2][:], vp[2][1])
        v2.passes(0, 8)
        v2.adds()
        sc(1, 7)
        x_one(8)
        av0(6)
        av0(7)
        sc(1, 8)
        av0(8)
        nc.vector.tensor_copy(nsb0[:], num0[:])
        nc.sync.dma_start(out=out_d[0:D + 1, :], in_=nsb0[:])
        for j in range(nj):
            av1(j)
    else:
        for j in range(4, nj):
            sc(0, j)
        q23 = ProjPair(qT128, HI, wqd, bq, q23s[:], NC)
        q23.passes(0, NE)
        q23.adds()
        for j in range(nj):
            sc(1, j)
        done_x = 0
        for i, (o, n) in enumerate(vp):
            pv(i)
            hi_j = nj if i == len(vp) - 1 else (o + n) // P
            for j in range(done_x, hi_j):
                x_one(j)
                av0(j)
            done_x = hi_j
        nc.vector.tensor_copy(nsb0[:], num0[:])
        nc.sync.dma_start(out=out_d[0:D + 1, :], in_=nsb0[:])
        for j in range(nj):
            av1(j)

    # tail: chunk c0 copy on ACT (idle after the last exp), c1 on DVE,
    # stores chunked so the first can fly while the second copies.
    nc.scalar.copy(nsb1[:, 0:NC], numc[0][:])
    nc.sync.dma_start(out=out_d[D + 1:2 * (D + 1), 0:NC],
                      in_=nsb1[:, 0:NC])
    nc.vector.tensor_copy(nsb1[:, NC:HI], numc[1][:])
    nc.sync.dma_start(out=out_d[D + 1:2 * (D + 1), NC:HI],
                      in_=nsb1[:, NC:HI])


_COMPILED = {}


def _get_compiled(sk2: int, nkr: int):
    key = (sk2, nkr)
    if key not in _COMPILED:
        nj = sk2 // P
        kp = _chunks(nkr, NC)
        vp = _chunks(nkr, NC)
        nc = bacc.Bacc("TRN2", target_bir_lowering=False, debug=False,
                       num_devices=N_CORES)

        def din(name, shape, dt=FP16):
            return nc.dram_tensor(name, shape, dt, kind="ExternalInput").ap()

        ins = {"wqd": din("wqd", [P, NE * P]),
               "wkv": din("wkv", [P, NE * P + NE * D]),
               "c32": din("c32", [P, nj + 3], F32),
               "q0": din("q0", [P, NE * NC]),
               "q1": din("q1", [P, NE * NC]),
               "q23": din("q23", [P, NE * 2 * NC])}
        for pref, pieces in (("k", kp), ("v", vp)):
            for i, (o, n) in enumerate(pieces):
                ins[f"{pref}{i}"] = din(f"{pref}{i}", [P, NE * n])
        out_d = nc.dram_tensor("out", [NH * (D + 1), HI], FP16,
                               kind="ExternalOutput").ap()
        with tile.TileContext(nc) as tc:
            with ExitStack() as ctx:
                _build(tc, ins, out_d, ctx, sk2, nkr)
        nc.compile()
        _COMPILED[key] = nc
    return _COMPILED[key]


def _blob(x16, lo, hi):
    """[S', E] fp16 row-slice -> staging blob [P, NE*(hi-lo)] laid out as
    [partition, e-block, col]."""
    return np.ascontiguousarray(
        x16[lo:hi].reshape(hi - lo, NE, P).transpose(2, 1, 0)
    ).reshape(P, -1)


LAST_RESULTS = None


def kernel(query, key, value, query_mask, key_mask, Wq, bq, Wk, bk, Wv, bv):
    global LAST_RESULTS
    query = np.asarray(query, dtype=np.float32)
    key = np.asarray(key, dtype=np.float32)
    value = np.asarray(value, dtype=np.float32)
    key_mask = np.asarray(key_mask)

    # compact masked keys away (they contribute exactly zero)
    keeps = [np.nonzero(key_mask[c] != 0)[0] for c in range(N_CORES)]
    nk_max = max(len(kps) for kps in keeps)
    sk2 = max(P, int(np.ceil(nk_max / P)) * P)
    sk2 = min(sk2, S)
    nkr = min(sk2, max(P, int(np.ceil(nk_max / 64)) * 64))
    nj = sk2 // P
    kp = _chunks(nkr, NC)
    vp = _chunks(nkr, NC)

    def wblob(w):
        return (np.asarray(w, np.float32).astype(np.float16)
                .reshape(D, NE, P).transpose(2, 1, 0).reshape(P, NE, D))

    wq3 = wblob(Wq)
    wqd = np.concatenate([wq3, wq3], axis=2).reshape(P, NE * P)
    wk3 = wblob(Wk)
    wkd = np.concatenate([wk3, wk3], axis=2).reshape(P, NE * P)
    wv2 = wblob(Wv).reshape(P, NE * D)
    wkv = np.ascontiguousarray(np.concatenate([wkd, wv2], axis=1))

    c32 = np.zeros((P, nj + 3), np.float32)
    for i, b in enumerate((bq, bk)):
        bb = np.asarray(b, np.float32).reshape(D)
        c32[0:D, nj + i] = bb
        c32[D:P, nj + i] = bb
    c32[0:D, nj + 2] = np.asarray(bv, np.float32).reshape(D)

    in_maps = []
    for c in range(N_CORES):
        kps = keeps[c]
        nk = len(kps)
        q16 = query[c].astype(np.float16)
        kc = np.zeros((nkr, E), np.float16)
        vc = np.zeros((nkr, E), np.float16)
        kc[0:nk] = key[c][kps].astype(np.float16)
        vc[0:nk] = value[c][kps].astype(np.float16)
        c32c = c32.copy()
        mbias = np.full(sk2, np.float32(MASK_NEG))
        mbias[0:nk] = 0.0
        c32c[:, 0:nj] = mbias.reshape(nj, P).T
        # q23 blob: [P, NE, 2*NC] with per-e layout [piece2 | piece3]
        b2 = _blob(q16, HI, HI + NC).reshape(P, NE, NC)
        b3 = _blob(q16, HI + NC, S).reshape(P, NE, NC)
        q23 = np.ascontiguousarray(
            np.concatenate([b2, b3], axis=2)).reshape(P, -1)
        im = {"wqd": wqd, "wkv": wkv, "c32": np.ascontiguousarray(c32c),
              "q0": _blob(q16, 0, NC), "q1": _blob(q16, NC, HI),
              "q23": q23}
        for pref, pieces, arr in (("k", kp, kc), ("v", vp, vc)):
            for i, (o, n) in enumerate(pieces):
                im[f"{pref}{i}"] = _blob(arr, o, o + n)
        in_maps.append(im)

    nc = _get_compiled(sk2, nkr)
    res = run_bass_kernel_spmd(nc, in_maps, core_ids=list(range(N_CORES)))
    LAST_RESULTS = res

    out = np.empty((N_CORES, S, D), np.float32)
    for c in range(N_CORES):
        o = np.asarray(res.results[c]["out"]).astype(np.float32)
        for h in range(NH):
            nh = o[h * (D + 1):(h + 1) * (D + 1)]
            out[c, h * HI:(h + 1) * HI] = (nh[0:D] / nh[D:D + 1]).T
    return out
